# revision 1
# baseline (speedup 1.0000x reference)
"""Causal self-attention block (QKV proj + RoPE + causal attention + o_proj)
on 8 Trainium2 NeuronCores.

Sharding: tensor-parallel over heads for QKV+attention (core c owns head
indices 4c..4c+3 of BOTH batch elements). The attention outputs are written
in an 8-shard layout; the host permutes shards (a pure gather, the moral
equivalent of an AllToAll - on-device collectives fail to load under this
runtime) so that launch 2 runs o_proj token-sharded: core c handles
(batch c//4, token block c%4) with the full o_proj weight. The final host
step is a pure concatenation of the 8 disjoint output slices.

Device notes:
- All GEMM operands in float16 (1 PE cycle/row; fp32 PSUM accumulation).
- q/k produced feature-major ([head_dim, tokens]) so attention scores need
  no transposes; v produced token-major for the PV matmul.
- Softmax computed in "transposed" orientation (scores^T [keys, queries]),
  max-free: Exp on ScalarE straight out of PSUM, key-dim sums via a
  ones-vector matmul, normalization via a rank-1 replicate matmul.
"""

import numpy as np

import concourse.bass as bass
import concourse.tile as tile
from concourse import bacc, mybir
from concourse.bass_utils import run_bass_kernel_spmd

HIDDEN = 4096
N_HEADS = 32
HEAD_DIM = 128
B, S = 2, 2048
T = B * S
N_CORES = 8
HPC = N_HEADS // N_CORES  # 4 head indices per core (both batches)
ROPE_BASE = 10000.0

FP = mybir.dt.float16
F32 = mybir.dt.float32

M_B = 1024               # token block in QKV GEMM
KC = HIDDEN // 128       # 32 contraction chunks
TQ = 512                 # query tile in attention
SCHUNKS = S // 128       # 16 key chunks per batch


def build_nc1():
    """QKV projection + RoPE + causal attention. Output: attention results
    in 8-shard layout attnT[s, :, :] = shard for (batch s//4, tokblock s%4),
    rows = this core's 4 heads x 128 dims, cols = 512 tokens."""
    nc = bacc.Bacc(num_devices=N_CORES, trn_type="TRN2")

    xT = nc.declare_dram_parameter("xT", [KC, 128, T], FP, isOutput=False)
    wqk = nc.declare_dram_parameter("wqk", [8, 128, KC, 128], FP, isOutput=False)
    wv = nc.declare_dram_parameter("wv", [KC, 128, 512], FP, isOutput=False)
    cosq = nc.declare_dram_parameter("cosq", [128, T], FP, isOutput=False)
    sinq = nc.declare_dram_parameter("sinq", [128, T], FP, isOutput=False)
    cosk = nc.declare_dram_parameter("cosk", [128, T], FP, isOutput=False)
    sink = nc.declare_dram_parameter("sink", [128, T], FP, isOutput=False)
    masks = nc.declare_dram_parameter("masks", [4, 128, TQ], FP, isOutput=False)
    onesc = nc.declare_dram_parameter("onesc", [128, 1], FP, isOutput=False)
    onesr = nc.declare_dram_parameter("onesr", [1, 128], FP, isOutput=False)
    attnT = nc.declare_dram_parameter("attnT", [8, 512, 512], F32, isOutput=True)

    qT_d = nc.dram_tensor("qT_d", [HPC, 128, T], FP)
    kT_d = nc.dram_tensor("kT_d", [HPC, 128, T], FP)
    v_d = nc.dram_tensor("v_d", [T, HPC * 128], FP)

    with nc.allow_low_precision(reason="float32r tiles carry full fp32 bits"), \
         tile.TileContext(nc) as tc:
        # ---------------- Phase A: QKV projection + RoPE ----------------
        with tc.tile_pool(name="xblk", bufs=2) as xpool, \
             tc.tile_pool(name="wq", bufs=2) as wqpool, \
             tc.tile_pool(name="wvp", bufs=3) as wvpool, \
             tc.tile_pool(name="rope", bufs=1) as rpool, \
             tc.tile_pool(name="ev", bufs=3) as evpool, \
             tc.tile_pool(name="psA", bufs=1, space="PSUM") as psA:
            for mb in range(T // M_B):
                t0 = mb * M_B
                xb = xpool.tile([128, KC, M_B], FP)
                nc.sync.dma_start(
                    xb[:], xT[:, :, t0:t0 + M_B].rearrange("kc p t -> p kc t"))

                # v: token-major [tok 128, feat 512], accumulate over kc.
                # Two half-passes of 512 tokens so v + q/k PSUM fits 8 banks.
                for half in range(M_B // 512):
                    h0 = t0 + half * 512
                    vps = []
                    for tt in range(4):
                        vps.append(psA.tile([128, 512], F32, tag=f"vps{tt}",
                                            bufs=1, name=f"vps{tt}"))
                    for kc in range(KC):
                        wvt = wvpool.tile([128, 512], FP, tag="wv")
                        nc.sync.dma_start(wvt[:], wv[kc])
                        for tt in range(4):
                            nc.tensor.matmul(
                                vps[tt][:],
                                xb[:, kc, half * 512 + tt * 128:
                                   half * 512 + (tt + 1) * 128], wvt[:],
                                start=(kc == 0), stop=(kc == KC - 1))
                    for tt in range(4):
                        vsb = evpool.tile([128, 512], FP, tag="vev")
                        nc.vector.tensor_copy(vsb[:], vps[tt][:])
                        nc.sync.dma_start(
                            v_d[h0 + tt * 128:h0 + (tt + 1) * 128, :], vsb[:])

                # q, k: feature-major [head_dim 128, tok 512] + fused RoPE
                cq = rpool.tile([128, M_B], FP, tag="cq")
                sq = rpool.tile([128, M_B], FP, tag="sq")
                ck = rpool.tile([128, M_B], FP, tag="ck")
                sk = rpool.tile([128, M_B], FP, tag="sk")
                nc.sync.dma_start(cq[:], cosq[:, t0:t0 + M_B])
                nc.sync.dma_start(sq[:], sinq[:, t0:t0 + M_B])
                nc.sync.dma_start(ck[:], cosk[:, t0:t0 + M_B])
                nc.sync.dma_start(sk[:], sink[:, t0:t0 + M_B])
                for ft in range(8):
                    # two 512-token PSUM tiles per f-tile: 1 LDWEIGHTS feeds
                    # 2 matmuls, hiding the weight load entirely
                    qpa = psA.tile([128, 512], F32, tag="qkpsa", bufs=2)
                    qpb = psA.tile([128, 512], F32, tag="qkpsb", bufs=2)
                    wt = wqpool.tile([128, KC, 128], FP, tag="wqk", bufs=2)
                    nc.sync.dma_start(wt[:], wqk[ft])
                    for kc in range(KC):
                        nc.tensor.matmul(qpa[:], wt[:, kc, :], xb[:, kc, 0:512],
                                         start=(kc == 0), stop=(kc == KC - 1))
                        nc.tensor.matmul(qpb[:], wt[:, kc, :], xb[:, kc, 512:1024],
                                         start=(kc == 0), stop=(kc == KC - 1))
                    is_q = ft < 4
                    h = ft % 4
                    cos2, sin2 = (cq, sq) if is_q else (ck, sk)
                    qsb = evpool.tile([128, M_B], FP, tag="qkev")
                    tmp = evpool.tile([128, M_B], FP, tag="ropetmp")
                    for hf, qps in ((0, qpa), (1, qpb)):
                        sl = slice(hf * 512, (hf + 1) * 512)
                        nc.vector.tensor_mul(qsb[:, sl], qps[:], cos2[:, sl])
                        nc.vector.tensor_mul(tmp[0:64, sl], qps[64:128, :],
                                             sin2[0:64, sl])
                        nc.vector.tensor_mul(tmp[64:128, sl], qps[0:64, :],
                                             sin2[64:128, sl])
                        nc.vector.tensor_sub(qsb[0:64, sl], qsb[0:64, sl],
                                             tmp[0:64, sl])
                        nc.vector.tensor_add(qsb[64:128, sl], qsb[64:128, sl],
                                             tmp[64:128, sl])
                    dst = qT_d if is_q else kT_d
                    nc.sync.dma_start(dst[h, :, t0:t0 + M_B], qsb[:])

        # ---------------- Phase B: causal attention ----------------
        with tc.tile_pool(name="const", bufs=1) as cpool, \
             tc.tile_pool(name="heads", bufs=2) as hpool, \
             tc.tile_pool(name="probs", bufs=4) as ppool, \
             tc.tile_pool(name="yt", bufs=3) as ypool, \
             tc.tile_pool(name="psB", bufs=1, space="PSUM") as psB:
            ones_col = cpool.tile([128, 1], FP)
            nc.sync.dma_start(ones_col[:], onesc[:])
            ones_row = cpool.tile([1, 128], FP)
            nc.sync.dma_start(ones_row[:], onesr[:])
            msk = cpool.tile([128, 2, 2, TQ], FP)
            nc.sync.dma_start(
                msk[:], masks.rearrange("(pi hf) p f -> p pi hf f", pi=2))

            for bb in range(B):
                for h in range(HPC):
                    qh = hpool.tile([128, S], FP, tag="qh")
                    kh = hpool.tile([128, S], FP, tag="kh")
                    vh = hpool.tile([128, SCHUNKS, 128], FP, tag="vh")
                    nc.sync.dma_start(qh[:], qT_d[h, :, bb * S:(bb + 1) * S])
                    nc.sync.dma_start(kh[:], kT_d[h, :, bb * S:(bb + 1) * S])
                    nc.sync.dma_start(
                        vh[:],
                        v_d[bb * S:(bb + 1) * S, h * 128:(h + 1) * 128]
                        .rearrange("(c p) d -> p c d", p=128))
                    for j in range(S // TQ):
                        nchunks = 4 * j + 4
                        npairs = nchunks // 2
                        aps = psB.tile([128, TQ], F32, tag="aps", bufs=2)
                        lps = psB.tile([1, TQ], F32, tag="lps", bufs=1)

                        def scores(p):
                            # two key chunks share one 2-bank PSUM tile so a
                            # single Exp covers 1024 columns
                            sps = psB.tile([128, 2, TQ], F32, tag="sps", bufs=2,
                                           name="sps")
                            for hf in range(2):
                                c = 2 * p + hf
                                nc.tensor.matmul(
                                    sps[:, hf, :], kh[:, c * 128:(c + 1) * 128],
                                    qh[:, j * TQ:(j + 1) * TQ],
                                    start=True, stop=True)
                            pr = ppool.tile([128, 2, TQ], FP, tag="pr", name="pr")
                            nc.scalar.activation(
                                pr[:], sps[:], mybir.ActivationFunctionType.Exp)
                            if 2 * p >= 4 * j:
                                nc.vector.tensor_mul(pr[:], pr[:],
                                                     msk[:, p - 2 * j, :, :])
                            return pr

                        def pv(p, pr):
                            for hf in range(2):
                                c = 2 * p + hf
                                nc.tensor.matmul(aps[:], vh[:, c, :], pr[:, hf, :],
                                                 start=(c == 0),
                                                 stop=(c == nchunks - 1))
                                nc.tensor.matmul(lps[:], ones_col[:], pr[:, hf, :],
                                                 start=(c == 0),
                                                 stop=(c == nchunks - 1))

                        # software pipeline: scores for pair p+1 are issued
                        # before the PV/sum matmuls of pair p, so the in-order
                        # PE has independent work while Exp(p) runs on ACT
                        prev = scores(0)
                        for p in range(1, npairs):
                            cur = scores(p)
                            pv(p - 1, prev)
                            prev = cur
                        pv(npairs - 1, prev)
                        r = ypool.tile([1, TQ], FP, tag="r")
                        nc.vector.tensor_copy(r[:], lps[:])
                        rps = psB.tile([128, TQ], F32, tag="rps", bufs=1)
                        nc.tensor.matmul(rps[:], ones_row[:], r[:], start=True, stop=True)
                        rrep = ypool.tile([128, TQ], F32, tag="rrep")
                        nc.vector.reciprocal_approx_fast(rrep[:], rps[:])
                        yt = ypool.tile([128, TQ], F32, tag="yt")
                        nc.vector.tensor_mul(yt[:], aps[:], rrep[:])
                        nc.sync.dma_start(
                            attnT[bb * 4 + j, h * 128:(h + 1) * 128, :], yt[:])

    nc.finalize()
    return nc


def build_nc2():
    """o_proj: out[of, tok] = sum_f WoT[f, of] * yT[f, tok] for this core's
    (batch, token-block) slice, with the full o_proj weight."""
    nc = bacc.Bacc(num_devices=N_CORES, trn_type="TRN2")
    yT = nc.declare_dram_parameter("yT", [KC, 128, 512], FP, isOutput=False)
    wo = nc.declare_dram_parameter("wo", [32, 128, KC, 128], FP, isOutput=False)
    out = nc.declare_dram_parameter("out", [HIDDEN, 512], F32, isOutput=True)

    with nc.allow_low_precision(reason="float32r tiles carry full fp32 bits"), \
         tile.TileContext(nc) as tc:
        with tc.tile_pool(name="yblk", bufs=1) as ybpool, \
             tc.tile_pool(name="wop", bufs=4) as wopool, \
             tc.tile_pool(name="oev", bufs=3) as oepool, \
             tc.tile_pool(name="psD", bufs=1, space="PSUM") as psD:
            ysb = ybpool.tile([128, KC, 512], FP)
            # chunked load so the first of-tiles can start before the whole
            # activation block has landed
            for kq in range(4):
                nc.sync.dma_start(
                    ysb[:, kq * 8:(kq + 1) * 8, :],
                    yT[kq * 8:(kq + 1) * 8].rearrange("kc p t -> p kc t"))
            for of in range(32):
                ops = psD.tile([128, 512], F32, tag="ops", bufs=4)
                wt = wopool.tile([128, KC, 128], FP, tag="wo", bufs=3)
                nc.sync.dma_start(wt[:], wo[of])
                for kc in range(KC):
                    nc.tensor.matmul(ops[:], wt[:, kc, :], ysb[:, kc, :],
                                     start=(kc == 0), stop=(kc == KC - 1))
                osb = oepool.tile([128, 512], F32, tag="oev")
                nc.vector.tensor_copy(osb[:], ops[:])
                nc.sync.dma_start(out[of * 128:(of + 1) * 128, :], osb[:])

    nc.finalize()
    return nc


_NC1 = None
_NC2 = None


def get_ncs():
    global _NC1, _NC2
    if _NC1 is None:
        _NC1 = build_nc1()
        _NC2 = build_nc2()
    return _NC1, _NC2


def _rope_tables(positions):
    """positions [B, S] int -> packed cos/sin tables [128, T] f32 in token
    order (b*S + t); rows [0:64] and [64:128] hold the same 64 freqs."""
    inv_freq = 1.0 / (ROPE_BASE ** (np.arange(0, HEAD_DIM, 2, dtype=np.float64)
                                    / HEAD_DIM))
    freqs = np.asarray(positions).reshape(T).astype(np.float64)[:, None] * inv_freq
    cos = np.cos(freqs).T.astype(np.float32)  # [64, T]
    sin = np.sin(freqs).T.astype(np.float32)
    cos2 = np.concatenate([cos, cos], axis=0)  # [128, T]
    sin2 = np.concatenate([sin, sin], axis=0)
    scale = np.float32(HEAD_DIM ** -0.5)
    return ((cos2 * scale).astype(np.float16), (sin2 * scale).astype(np.float16),
            cos2.astype(np.float16), sin2.astype(np.float16))


def prepare_inputs1(hidden_states, positions, W_pack):
    x = np.ascontiguousarray(np.asarray(hidden_states, dtype=np.float32)
                             .reshape(T, HIDDEN))
    xT_blocks = np.ascontiguousarray(x.T.astype(np.float16)).reshape(KC, 128, T)

    cosq, sinq, cosk, sink = _rope_tables(positions)

    mk = np.zeros((4, 128, TQ), dtype=np.float16)
    p = np.arange(128)[:, None]
    f = np.arange(TQ)[None, :]
    for d in range(4):
        mk[d] = (p + 128 * d <= f).astype(np.float16)

    in_maps = []
    for c in range(N_CORES):
        hs = [HPC * c + i for i in range(HPC)]
        wqk_blocks = np.empty((8, 128, KC, 128), dtype=np.float16)
        for ft in range(8):
            off = 0 if ft < 4 else HIDDEN
            h = hs[ft % 4]
            wsl = W_pack[off + h * 128: off + (h + 1) * 128, :]  # [128, 4096]
            wqk_blocks[ft] = wsl.reshape(128, KC, 128).transpose(2, 1, 0)
        wv_sl = np.concatenate(
            [W_pack[2 * HIDDEN + h * 128: 2 * HIDDEN + (h + 1) * 128, :]
             for h in hs], axis=0)  # [512, 4096]
        wv_blocks = np.ascontiguousarray(
            wv_sl.astype(np.float16).reshape(512, KC, 128).transpose(1, 2, 0))
        in_maps.append({
            "xT": xT_blocks,
            "wqk": np.ascontiguousarray(wqk_blocks),
            "wv": wv_blocks,
            "cosq": cosq, "sinq": sinq, "cosk": cosk, "sink": sink,
            "masks": mk,
            "onesc": np.ones((128, 1), dtype=np.float16),
            "onesr": np.ones((1, 128), dtype=np.float16),
        })
    return in_maps


def kernel(hidden_states, positions, W_pack, W_o):
    W_pack = np.asarray(W_pack, dtype=np.float32)
    W_o = np.asarray(W_o, dtype=np.float32)

    nc1, nc2 = get_ncs()
    in_maps1 = prepare_inputs1(hidden_states, positions, W_pack)
    res1 = run_bass_kernel_spmd(nc1, in_maps1, list(range(N_CORES)))

    # Host-side shard permutation (the "AllToAll"): pure gather, no math.
    wo_blocks = np.ascontiguousarray(
        np.ascontiguousarray(W_o.T.astype(np.float16))
        .reshape(KC, 128, 32, 128).transpose(2, 1, 0, 3))
    in_maps2 = []
    for j in range(N_CORES):
        yT = np.concatenate([res1.results[c]["attnT"][j] for c in range(N_CORES)],
                            axis=0)  # [4096, 512] feature-major, head order
        in_maps2.append({"yT": yT.astype(np.float16).reshape(KC, 128, 512),
                         "wo": wo_blocks})
    res2 = run_bass_kernel_spmd(nc2, in_maps2, list(range(N_CORES)))

    out = np.empty((B, S, HIDDEN), dtype=np.float32)
    for c in range(N_CORES):
        bb, j = c // 4, c % 4
        out[bb, j * 512:(j + 1) * 512, :] = res2.results[c]["out"].T
    return out

